# revision 11
# baseline (speedup 1.0000x reference)
"""EdgeEmbedding forward on 8 Trainium2 NeuronCores.

Computation (see reference):
    type_attr_sum[t] = sum_{j: attr_seg_ids[j]==t} attr_table[flat_attr_ids[j]]
    combined[t]      = edge_type_embedding[t] + type_attr_sum[t]        # [1000, 256]
    out[i]           = combined[data[i]]                                # [1M, 256]

Distribution / algorithm (zero-collective, type-sharded, type-sorted):
  Core k owns types [125k, 125k+125) and computes ALL edges whose type
  falls in its range (host buckets the 1M edges by type//125; counts are
  ~125K +- 350 per core, padded to 126976 static slots).  No AllReduce /
  AllGather / barrier exists in the program - cores run fully
  independently; the host does all shard/unshard index marshaling.

  Stage 1 (segment sum): the host dedups the attr rows referenced by the
  core's types (~6.2K unique rows), ships them as a compact row table U
  [6912, 256] bf16 plus a count matrix M[t, c] = #refs of type t hitting
  compact row c.  The device computes type_attr_sum = M @ U as 54
  accumulating PE matmuls, adds the core's edge_type_embedding slice
  (f32), and casts to fp16 -> chi [128, 256].  No SWDGE gather at all.

  Stage 2 (edge gather): the host SORTS the core's edges by type, so
  each type's edges form one contiguous run [S_t, E_t) of slots.  The
  one-hot tile oh[t, e] = (e >= S_t) & (e < E_t) is built per 2048-slot
  half-seg from a constant fp16 iota row (values 0..2047, exact in
  fp16) and per-partition start/end scalar columns - two fused
  all-16-bit DVE ops (tensor_scalar is_ge + scalar_tensor_tensor
  is_lt*mult) at 2x rate, with ZERO per-edge HBM index traffic (the
  16MB replicated dval tensor of earlier versions is gone).  32 PE
  matmuls per seg emit PSUM[e, :] = oh.T @ chi; PSUM is evacuated
  f32->fp16 in 6/2-split copies on ACT/DVE, staged so each SBUF
  partition holds 32 CONSECUTIVE output rows -> every output DMA packet
  is 16KB contiguous (~25GB/s per DMA engine).  The one-hot build for
  seg s+1 is emitted BEFORE the evac casts of seg s so the strict-FIFO
  DVE queue never head-of-line blocks the PE.  The host inverse-permutes
  rows on unshard.
"""
import sys

sys.path.insert(0, "/opt/trn_rl_repo")

import numpy as np

import concourse.bass as bass
import concourse.bacc as bacc
import concourse.mybir as mybir
from concourse.tile import TileContext
from concourse.bass_utils import run_bass_kernel_spmd

# ---- problem constants (hardcoded per harness contract) ----
N = 1_000_000
D = 256
NSEG = 1000
NCORES = 8
TPC = NSEG // NCORES        # 125 types per core

NU_PAD = 6912               # compact attr-row slots (54 Ktiles; max used ~6256)
KT1 = NU_PAD // 128         # 54 stage-1 matmul tiles

L = 126_976                 # padded edge slots per core (992 Ktiles; max used ~125392)
SEG = 4096                  # edges per matmul/evac/DMA granule
NSEGS = L // SEG            # 31
HS = 2048                   # half-seg: fp16 iota stays integer-exact
NH = L // HS                # 62
TILES_PER_SEG = SEG // 128  # 32
GROUP = 32                  # edge tiles per output DMA (2MB, 16KB/partition)

_cached = {}


def _build_program():
    if "nc" in _cached:
        return _cached["nc"]
    nc = bacc.Bacc("TRN2", target_bir_lowering=False, debug=False, num_devices=NCORES)

    f32 = mybir.dt.float32
    bf16 = mybir.dt.bfloat16
    fp16 = mybir.dt.float16

    # stage-1 tables: U rows interleaved so row r sits at [r%128, r//128, :]
    u_tab = nc.dram_tensor("u_tab", [128, KT1 * D], bf16, kind="ExternalInput")
    m_tab = nc.dram_tensor("m_tab", [128, KT1 * 128], bf16, kind="ExternalInput")
    emb = nc.dram_tensor("emb", [128, D], f32, kind="ExternalInput")
    iota = nc.dram_tensor("iota", [128, HS], fp16, kind="ExternalInput")
    se_s = nc.dram_tensor("se_s", [128, NH], f32, kind="ExternalInput")
    se_e = nc.dram_tensor("se_e", [128, NH], f32, kind="ExternalInput")
    out_dev = nc.dram_tensor("out_dev", [L, D], fp16, kind="ExternalOutput")

    with TileContext(nc) as tc:
        with (
            tc.tile_pool(name="misc", bufs=1) as misc,
            tc.tile_pool(name="s2t1", bufs=3) as s2t1,
            tc.tile_pool(name="s2oh", bufs=3) as s2oh,
            tc.tile_pool(name="s2ps", bufs=2, space="PSUM") as s2ps,
            tc.tile_pool(name="s2st", bufs=4) as s2st,
        ):
            # ---- prologue: stage-1 tables (halved loads so matmuls start early) ----
            H = KT1 // 2
            u_sb = misc.tile([128, KT1, D], bf16)
            m_sb = misc.tile([128, KT1, 128], bf16)
            nc.sync.dma_start(out=m_sb[:, :H, :], in_=m_tab.ap()[:, :H * 128])
            nc.sync.dma_start(out=u_sb[:, :H, :], in_=u_tab.ap()[:, :H * D])
            nc.sync.dma_start(out=m_sb[:, H:, :], in_=m_tab.ap()[:, H * 128:])
            nc.sync.dma_start(out=u_sb[:, H:, :], in_=u_tab.ap()[:, H * D:])
            emb_sb = misc.tile([128, D], f32)
            nc.sync.dma_start(out=emb_sb[:, :], in_=emb.ap())
            iota_t = misc.tile([128, HS], fp16)
            nc.gpsimd.dma_start(out=iota_t[:, :], in_=iota.ap())
            ses_t = misc.tile([128, NH], f32)
            nc.gpsimd.dma_start(out=ses_t[:, :], in_=se_s.ap())
            see_t = misc.tile([128, NH], f32)
            nc.gpsimd.dma_start(out=see_t[:, :], in_=se_e.ap())

            def build_oh(s):
                # oh[t, e] = (e >= S_t) & (e < E_t) per half-seg, all-fp16 2x ops
                oh = s2oh.tile([128, SEG], fp16, tag="oh", name=f"oh{s}")
                for half in range(2):
                    hh = 2 * s + half
                    t1 = s2t1.tile([128, HS], fp16, tag="t1", name=f"t1_{s}_{half}")
                    nc.vector.tensor_scalar(
                        t1[:, :], iota_t[:, :], ses_t[:, hh:hh + 1], None,
                        op0=mybir.AluOpType.is_ge,
                    )
                    nc.vector.scalar_tensor_tensor(
                        oh[:, half * HS:(half + 1) * HS],
                        iota_t[:, :], see_t[:, hh:hh + 1], t1[:, :],
                        op0=mybir.AluOpType.is_lt,
                        op1=mybir.AluOpType.mult,
                    )
                return oh

            # ---- stage 1: type_attr_sum = M @ U (accumulating bf16 matmul) ----
            # borrows a stage-2 PSUM tile; all 8 banks stay with the s2ps pool
            pse = s2ps.tile([128, 8, D], f32, tag="pp", name="ps1")
            ps1 = pse[:, 0, :]
            for j in range(KT1):
                nc.tensor.matmul(
                    ps1, m_sb[:, j, :], u_sb[:, j, :],
                    start=(j == 0), stop=(j == KT1 - 1),
                )
            # chi = fp16(ps1 + emb) in one DVE op (reads PSUM + SBUF)
            chi = misc.tile([128, D], fp16, tag="chi")
            nc.vector.tensor_tensor(
                chi[:, :], ps1, emb_sb[:, :], op=mybir.AluOpType.add,
            )

            # ---- stage 2: software-pipelined over segs ----
            oh_cur = build_oh(0)
            for s in range(NSEGS):
                # prefetch next seg's one-hot BEFORE this seg's evac casts so
                # the FIFO DVE queue can't block the PE
                oh_next = build_oh(s + 1) if s + 1 < NSEGS else None
                st = s2st.tile([128, GROUP, D], fp16, tag="st", name=f"st{s}")
                for g in range(TILES_PER_SEG // 8):
                    pp = s2ps.tile([128, 8, D], f32, tag="pp", name=f"pp{s}_{g}")
                    for h in range(8):
                        u = g * 8 + h
                        nc.tensor.matmul(
                            pp[:, h, :],
                            oh_cur[:, u * 128:(u + 1) * 128],
                            chi[:, :],
                            start=True, stop=True,
                        )
                    # evacuate PSUM f32 -> SBUF fp16 (GPSIMD can't read PSUM)
                    na = 6 if g % 2 == 0 else 7
                    nc.scalar.copy(st[:, g * 8:g * 8 + na, :], pp[:, :na, :])
                    nc.vector.tensor_copy(st[:, g * 8 + na:g * 8 + 8, :], pp[:, na:, :])
                # partition p holds output rows s*4096 + 32p .. 32p+31
                dst = bass.AP(out_dev, s * SEG * D, [[GROUP * D, 128], [D, GROUP], [1, D]])
                nc.sync.dma_start(out=dst, in_=st[:, :, :])
                oh_cur = oh_next

    nc.compile()
    _cached["nc"] = nc
    return nc


def _prep_in_maps(data, attr_table, edge_type_embedding, flat_attr_ids, attr_seg_ids):
    import ml_dtypes
    bf16 = ml_dtypes.bfloat16

    ids = np.asarray(flat_attr_ids).astype(np.int64)
    segs = np.asarray(attr_seg_ids).astype(np.int64)
    data = np.asarray(data).astype(np.int64)
    attr_table = np.ascontiguousarray(np.asarray(attr_table, dtype=np.float32))
    edge_emb = np.asarray(edge_type_embedding, dtype=np.float32)

    iota = np.broadcast_to(np.arange(HS, dtype=np.float16)[None, :], (128, HS))

    in_maps = []
    edge_perm = []
    for k in range(NCORES):
        # ---- stage 1: compact dedup table + count matrix for this core ----
        own = (segs // TPC) == k
        tloc = segs[own] - TPC * k                       # 0..124
        uniq, inv = np.unique(ids[own], return_inverse=True)
        nu = len(uniq)
        assert nu <= NU_PAD, f"core {k}: {nu} unique attr rows > {NU_PAD}"
        U = np.zeros((NU_PAD, D), np.float32)
        U[:nu] = attr_table[uniq]
        M = np.zeros((128, NU_PAD), np.float32)
        np.add.at(M, (tloc, inv), 1.0)
        # interleave for device tiles: row r -> [r%128, r//128]
        u_tab = np.ascontiguousarray(
            U.reshape(KT1, 128, D).transpose(1, 0, 2).reshape(128, KT1 * D).astype(bf16))
        # lhsT tile j needs [c_part, t_free] = M[t, 128j + c]
        m_tab = np.ascontiguousarray(
            M.T.reshape(KT1, 128, 128).transpose(1, 0, 2).reshape(128, KT1 * 128).astype(bf16))

        emb_k = np.zeros((128, D), np.float32)
        emb_k[:TPC] = edge_emb[TPC * k:TPC * (k + 1)]

        # ---- stage 2: this core's edges, SORTED by type, pads at the end ----
        sel = np.nonzero((data // TPC) == k)[0]
        nk = sel.shape[0]
        assert nk <= L, f"core {k}: {nk} edges > {L}"
        dv = (data[sel] - TPC * k).astype(np.int64)      # 0..124
        order = np.argsort(dv, kind="stable")
        edge_perm.append(sel[order])                     # slot p holds edge sel[order[p]]
        runs = np.zeros(126, np.int64)
        np.add.at(runs, dv + 1, 1)
        bounds = np.cumsum(runs)                         # S_t = bounds[t], E_t = bounds[t+1]
        S = np.zeros(128, np.int64)
        E = np.zeros(128, np.int64)
        S[:TPC] = bounds[:TPC]
        E[:TPC] = bounds[1:TPC + 1]
        # per half-seg clipped boundaries (f32; values 0..2048 exact)
        h0 = np.arange(NH, dtype=np.int64)[None, :] * HS
        se_s = np.clip(S[:, None] - h0, 0, HS).astype(np.float32)
        se_e = np.clip(E[:, None] - h0, 0, HS).astype(np.float32)

        in_maps.append({
            "u_tab": u_tab,
            "m_tab": m_tab,
            "emb": emb_k,
            "iota": np.ascontiguousarray(iota),
            "se_s": np.ascontiguousarray(se_s),
            "se_e": np.ascontiguousarray(se_e),
        })

    # slot s -> HBM row: (s//4096)*4096 + (s%128)*32 + (s%4096)//128
    s = np.arange(L, dtype=np.int64)
    slot2row = (s // SEG) * SEG + (s % 128) * GROUP + (s % SEG) // 128
    return in_maps, edge_perm, slot2row


def run(inputs, trace=False, trace_cores=None):
    nc = _build_program()
    in_maps, edge_perm, slot2row = _prep_in_maps(**inputs)
    kwargs = {}
    if trace:
        kwargs = dict(trace=True)
        if trace_cores is not None:
            kwargs["trace_cores"] = trace_cores
    res = run_bass_kernel_spmd(nc, in_maps, core_ids=list(range(NCORES)), **kwargs)
    outp = np.empty((N, D), np.float32)
    for k in range(NCORES):
        perm = edge_perm[k]                              # global edge idx per slot
        rows = slot2row[:perm.shape[0]]
        outp[perm] = res.results[k]["out_dev"][rows].astype(np.float32)
    return outp, res


def kernel(**inputs) -> np.ndarray:
    outp, _ = run(inputs, trace=False)
    return outp


# revision 13
# speedup vs baseline: 1.3087x; 1.3087x over previous
"""EdgeEmbedding forward on 8 Trainium2 NeuronCores.

Computation (see reference):
    type_attr_sum[t] = sum_{j: attr_seg_ids[j]==t} attr_table[flat_attr_ids[j]]
    combined[t]      = edge_type_embedding[t] + type_attr_sum[t]        # [1000, 256]
    out[i]           = combined[data[i]]                                # [1M, 256]

Distribution / algorithm (zero-collective, type-sharded, block-replicated):
  Core k owns types [125k, 125k+125) and computes ALL edges whose type
  falls in its range (host buckets the 1M edges by type//125, ~125K
  edges/core).  No collectives exist in the program - cores run fully
  independently; the host does all shard/unshard index marshaling.

  Stage 1 (segment sum): the host dedups the attr rows referenced by the
  core's types (~6.2K unique rows), ships them as a compact row table U
  [6912, 256] bf16 plus a count matrix M[t, c] = #refs of type t hitting
  compact row c.  The device computes type_attr_sum = M @ U as 54
  accumulating PE matmuls, adds the core's edge_type_embedding slice
  (f32), and casts to fp16 -> chi [128, 256].

  Stage 2 (edge gather): out[i] = chi[type(i)] is PIECEWISE-CONSTANT
  once edges are sorted by type.  The host sorts the core's edges and
  pads each type's run to a multiple of 32 rows, so the 131072 output
  rows form 4096 single-type 32-row blocks.  The device gathers ONE row
  per block: a block one-hot ohb[t, b] = (b >= Sb_t) & (b < Eb_t) is
  built from a constant fp16 iota and per-partition run-boundary
  scalars (6 cheap DVE ops total), 32 PE matmuls emit the 4096 block
  rows into PSUM, ACT/DVE evacuate them fp16 into a [128, 32, 256]
  block table, and 32 output DMAs REPLICATE each block row 32x via a
  stride-0 source dimension (each DMA: partition p's single 512B block
  row -> 16KB of contiguous HBM).  Total non-DMA work is ~32x smaller
  than gathering every edge row through the PE; the kernel is
  essentially the 67MB HBM output write.  The host maps each edge to
  row S'_type + rank on unshard and drops pad rows.
"""
import sys

sys.path.insert(0, "/opt/trn_rl_repo")

import numpy as np

import concourse.bass as bass
import concourse.bacc as bacc
import concourse.mybir as mybir
from concourse.tile import TileContext
from concourse.bass_utils import run_bass_kernel_spmd

# ---- problem constants (hardcoded per harness contract) ----
N = 1_000_000
D = 256
NSEG = 1000
NCORES = 8
TPC = NSEG // NCORES        # 125 types per core

NU_PAD = 6912               # compact attr-row slots (54 Ktiles; max used ~6256)
KT1 = NU_PAD // 128         # 54 stage-1 matmul tiles

BLK = 32                    # rows per single-type output block
NB = 4096                   # block slots (>= max used ~4040)
L = NB * BLK                # 131072 padded output rows per core
HS = 2048                   # half of NB: fp16 iota stays integer-exact
NTILE = NB // 128           # 32 block-gather matmul tiles

_cached = {}


def _build_program():
    if "nc" in _cached:
        return _cached["nc"]
    nc = bacc.Bacc("TRN2", target_bir_lowering=False, debug=False, num_devices=NCORES)

    f32 = mybir.dt.float32
    bf16 = mybir.dt.bfloat16
    fp16 = mybir.dt.float16

    # stage-1 tables: U rows interleaved so row r sits at [r%128, r//128, :]
    u_tab = nc.dram_tensor("u_tab", [128, KT1 * D], bf16, kind="ExternalInput")
    m_tab = nc.dram_tensor("m_tab", [128, KT1 * 128], bf16, kind="ExternalInput")
    emb = nc.dram_tensor("emb", [128, D], f32, kind="ExternalInput")
    iota = nc.dram_tensor("iota", [128, HS], fp16, kind="ExternalInput")
    se_s = nc.dram_tensor("se_s", [128, 2], f32, kind="ExternalInput")
    se_e = nc.dram_tensor("se_e", [128, 2], f32, kind="ExternalInput")
    out_dev = nc.dram_tensor("out_dev", [L, D], fp16, kind="ExternalOutput")

    with TileContext(nc) as tc:
        with (
            tc.tile_pool(name="misc", bufs=1) as misc,
            tc.tile_pool(name="s2ps", bufs=2, space="PSUM") as s2ps,
        ):
            # ---- prologue ----
            u_sb = misc.tile([128, KT1, D], bf16)
            m_sb = misc.tile([128, KT1, 128], bf16)
            qs = [0, 14, 28, 41, KT1]
            for qi in range(4):
                lo, hi = qs[qi], qs[qi + 1]
                nc.sync.dma_start(out=m_sb[:, lo:hi, :], in_=m_tab.ap()[:, lo * 128:hi * 128])
                nc.sync.dma_start(out=u_sb[:, lo:hi, :], in_=u_tab.ap()[:, lo * D:hi * D])
            emb_sb = misc.tile([128, D], f32)
            nc.scalar.dma_start(out=emb_sb[:, :], in_=emb.ap())
            iota_t = misc.tile([128, HS], fp16)
            nc.scalar.dma_start(out=iota_t[:, :], in_=iota.ap())
            ses_t = misc.tile([128, 2], f32)
            nc.scalar.dma_start(out=ses_t[:, :], in_=se_s.ap())
            see_t = misc.tile([128, 2], f32)
            nc.scalar.dma_start(out=see_t[:, :], in_=se_e.ap())

            # ---- block one-hot: ohb[t, b] = (b >= Sb_t) & (b < Eb_t) ----
            ohb = misc.tile([128, NB], fp16, tag="ohb")
            for half in range(2):
                t1 = misc.tile([128, HS], fp16, tag=f"t1_{half}")
                nc.vector.tensor_scalar(
                    t1[:, :], iota_t[:, :], ses_t[:, half:half + 1], None,
                    op0=mybir.AluOpType.is_ge,
                )
                t2 = misc.tile([128, HS], fp16, tag=f"t2_{half}")
                nc.vector.tensor_scalar(
                    t2[:, :], iota_t[:, :], see_t[:, half:half + 1], None,
                    op0=mybir.AluOpType.is_ge,
                )
                nc.vector.tensor_tensor(
                    ohb[:, half * HS:(half + 1) * HS], t1[:, :], t2[:, :],
                    op=mybir.AluOpType.subtract,
                )

            # ---- stage 1: type_attr_sum = M @ U (accumulating bf16 matmul) ----
            pse = s2ps.tile([128, 8, D], f32, tag="pp", name="ps1")
            ps1 = pse[:, 0, :]
            for j in range(KT1):
                nc.tensor.matmul(
                    ps1, m_sb[:, j, :], u_sb[:, j, :],
                    start=(j == 0), stop=(j == KT1 - 1),
                )
            chi = misc.tile([128, D], fp16, tag="chi")
            nc.vector.tensor_tensor(
                chi[:, :], ps1, emb_sb[:, :], op=mybir.AluOpType.add,
            )

            # ---- stage 2: gather one row per block, then replicate via DMA ----
            bt = misc.tile([128, NTILE, D], fp16, tag="bt")
            for g in range(NTILE // 8):
                pp = s2ps.tile([128, 8, D], f32, tag="pp", name=f"pp{g}")
                for h in range(8):
                    u = g * 8 + h
                    nc.tensor.matmul(
                        pp[:, h, :],
                        ohb[:, u * 128:(u + 1) * 128],
                        chi[:, :],
                        start=True, stop=True,
                    )
                nc.scalar.copy(bt[:, g * 8:g * 8 + 5, :], pp[:, :5, :])
                nc.vector.tensor_copy(bt[:, g * 8 + 5:g * 8 + 8, :], pp[:, 5:, :])

            # 32 output DMAs: block b = j*128 + p -> HBM rows 32b..32b+31,
            # all 32 rows replicated from bt[p, j, :] via a stride-0 src dim
            for j in range(NTILE):
                v = bt[:, j, :]
                src_ap = bass.AP(v.tensor, v.offset, [[v.ap[0][0], 128], [0, BLK], [1, D]])
                dst_ap = bass.AP(out_dev, j * 128 * BLK * D,
                                 [[BLK * D, 128], [D, BLK], [1, D]])
                nc.sync.dma_start(out=dst_ap, in_=src_ap)

    nc.compile()
    _cached["nc"] = nc
    return nc


def _prep_in_maps(data, attr_table, edge_type_embedding, flat_attr_ids, attr_seg_ids):
    import ml_dtypes
    bf16 = ml_dtypes.bfloat16

    ids = np.asarray(flat_attr_ids).astype(np.int64)
    segs = np.asarray(attr_seg_ids).astype(np.int64)
    data = np.asarray(data).astype(np.int64)
    attr_table = np.ascontiguousarray(np.asarray(attr_table, dtype=np.float32))
    edge_emb = np.asarray(edge_type_embedding, dtype=np.float32)

    iota = np.broadcast_to(np.arange(HS, dtype=np.float16)[None, :], (128, HS))

    in_maps = []
    edge_rows = []
    for k in range(NCORES):
        # ---- stage 1: compact dedup table + count matrix for this core ----
        own = (segs // TPC) == k
        tloc = segs[own] - TPC * k                       # 0..124
        uniq, inv = np.unique(ids[own], return_inverse=True)
        nu = len(uniq)
        assert nu <= NU_PAD, f"core {k}: {nu} unique attr rows > {NU_PAD}"
        U = np.zeros((NU_PAD, D), np.float32)
        U[:nu] = attr_table[uniq]
        M = np.zeros((128, NU_PAD), np.float32)
        np.add.at(M, (tloc, inv), 1.0)
        u_tab = np.ascontiguousarray(
            U.reshape(KT1, 128, D).transpose(1, 0, 2).reshape(128, KT1 * D).astype(bf16))
        m_tab = np.ascontiguousarray(
            M.T.reshape(KT1, 128, 128).transpose(1, 0, 2).reshape(128, KT1 * 128).astype(bf16))

        emb_k = np.zeros((128, D), np.float32)
        emb_k[:TPC] = edge_emb[TPC * k:TPC * (k + 1)]

        # ---- stage 2: 32-row single-type blocks ----
        sel = np.nonzero((data // TPC) == k)[0]
        dv = (data[sel] - TPC * k).astype(np.int64)      # 0..124
        cnt = np.bincount(dv, minlength=TPC)             # edges per type
        nblk = (cnt + BLK - 1) // BLK                    # blocks per type
        assert nblk.sum() <= NB, f"core {k}: {nblk.sum()} blocks > {NB}"
        Sb = np.zeros(128, np.int64)
        Eb = np.zeros(128, np.int64)
        Sb[:TPC] = np.concatenate(([0], np.cumsum(nblk)[:-1]))
        Eb[:TPC] = np.cumsum(nblk)
        # edge -> output row: padded run start + rank within type
        run_start = Sb[:TPC] * BLK                       # padded row starts
        cum = np.concatenate(([0], np.cumsum(cnt)[:-1]))  # unpadded starts
        order = np.argsort(dv, kind="stable")
        ranks = np.arange(dv.shape[0]) - cum[dv[order]]  # rank within type (sorted)
        rows = np.empty(dv.shape[0], np.int64)
        rows[order] = run_start[dv[order]] + ranks
        edge_rows.append((sel, rows))

        se_s_arr = np.stack([np.clip(Sb, 0, HS), np.clip(Sb - HS, 0, HS)], axis=1)
        se_e_arr = np.stack([np.clip(Eb, 0, HS), np.clip(Eb - HS, 0, HS)], axis=1)

        in_maps.append({
            "u_tab": u_tab,
            "m_tab": m_tab,
            "emb": emb_k,
            "iota": np.ascontiguousarray(iota),
            "se_s": np.ascontiguousarray(se_s_arr.astype(np.float32)),
            "se_e": np.ascontiguousarray(se_e_arr.astype(np.float32)),
        })
    return in_maps, edge_rows


def run(inputs, trace=False, trace_cores=None):
    nc = _build_program()
    in_maps, edge_rows = _prep_in_maps(**inputs)
    kwargs = {}
    if trace:
        kwargs = dict(trace=True)
        if trace_cores is not None:
            kwargs["trace_cores"] = trace_cores
    res = run_bass_kernel_spmd(nc, in_maps, core_ids=list(range(NCORES)), **kwargs)
    outp = np.empty((N, D), np.float32)
    for k in range(NCORES):
        sel, rows = edge_rows[k]
        outp[sel] = res.results[k]["out_dev"][rows].astype(np.float32)
    return outp, res


def kernel(**inputs) -> np.ndarray:
    outp, _ = run(inputs, trace=False)
    return outp


# revision 15
# speedup vs baseline: 1.7784x; 1.3589x over previous
"""EdgeEmbedding forward on 8 Trainium2 NeuronCores.

Computation (see reference):
    type_attr_sum[t] = sum_{j: attr_seg_ids[j]==t} attr_table[flat_attr_ids[j]]
    combined[t]      = edge_type_embedding[t] + type_attr_sum[t]        # [1000, 256]
    out[i]           = combined[data[i]]                                # [1M, 256]

Distribution / algorithm (zero-collective, type-sharded, block-replicated):
  Core k owns types [125k, 125k+125) and computes ALL edges whose type
  falls in its range (host buckets the 1M edges by type//125, ~125K
  edges/core).  No collectives exist in the program - cores run fully
  independently; the host does all shard/unshard index marshaling.

  Stage 1 (segment sum): the host dedups the attr rows referenced by the
  core's types (~6.2K unique rows), ships them as a compact row table U
  [6912, 256] bf16 plus a count matrix M[t, c] = #refs of type t hitting
  compact row c.  The device computes type_attr_sum = M @ U as 54
  accumulating PE matmuls, adds the core's edge_type_embedding slice
  (f32), and casts to fp16 -> chi [128, 256].

  Stage 2 (edge gather): out[i] = chi[type(i)] is PIECEWISE-CONSTANT
  once edges are sorted by type.  The host sorts the core's edges and
  pads each type's run to a multiple of 32 rows, so the 131072 output
  rows form 4096 single-type 32-row blocks.  The device gathers ONE row
  per block: a block one-hot ohb[t, b] = (b >= Sb_t) & (b < Eb_t) is
  built from a constant fp16 iota and per-partition run-boundary
  scalars (6 cheap DVE ops total), 32 PE matmuls emit the 4096 block
  rows into PSUM, ACT/DVE evacuate them fp16 into a [128, 32, 256]
  block table, and 32 output DMAs REPLICATE each block row 32x via a
  stride-0 source dimension (each DMA: partition p's single 512B block
  row -> 16KB of contiguous HBM).  Total non-DMA work is ~32x smaller
  than gathering every edge row through the PE; the kernel is
  essentially the 67MB HBM output write.  The host maps each edge to
  row S'_type + rank on unshard and drops pad rows.
"""
import sys

sys.path.insert(0, "/opt/trn_rl_repo")

import numpy as np

import concourse.bass as bass
import concourse.bacc as bacc
import concourse.mybir as mybir
from concourse.tile import TileContext
from concourse.bass_utils import run_bass_kernel_spmd

# ---- problem constants (hardcoded per harness contract) ----
N = 1_000_000
D = 256
NSEG = 1000
NCORES = 8
TPC = NSEG // NCORES        # 125 types per core

NU_PAD = 6912               # compact attr-row slots (54 Ktiles; max used ~6256)
KT1 = NU_PAD // 128         # 54 stage-1 matmul tiles

BLK = 32                    # rows per single-type output block
NB = 4096                   # block slots (>= max used ~4040)
L = NB * BLK                # 131072 padded output rows per core
HS = 2048                   # half of NB: fp16 iota stays integer-exact
NTILE = NB // 128           # 32 block-gather matmul tiles

_cached = {}


def _build_program():
    if "nc" in _cached:
        return _cached["nc"]
    nc = bacc.Bacc("TRN2", target_bir_lowering=False, debug=False, num_devices=NCORES)

    f32 = mybir.dt.float32
    bf16 = mybir.dt.bfloat16
    fp16 = mybir.dt.float16

    # stage-1 tables: U rows interleaved so row r sits at [r%128, r//128, :]
    u_tab = nc.dram_tensor("u_tab", [128, KT1 * D], bf16, kind="ExternalInput")
    m_tab = nc.dram_tensor("m_tab", [128, KT1 * 128], bf16, kind="ExternalInput")
    emb = nc.dram_tensor("emb", [128, D], f32, kind="ExternalInput")
    iota = nc.dram_tensor("iota", [128, HS], fp16, kind="ExternalInput")
    se_s = nc.dram_tensor("se_s", [128, 2], f32, kind="ExternalInput")
    se_e = nc.dram_tensor("se_e", [128, 2], f32, kind="ExternalInput")
    out_dev = nc.dram_tensor("out_dev", [L, D], fp16, kind="ExternalOutput")

    with TileContext(nc) as tc:
        with (
            tc.tile_pool(name="misc", bufs=1) as misc,
            tc.tile_pool(name="s2ps", bufs=2, space="PSUM") as s2ps,
            tc.tile_pool(name="s2st", bufs=4) as s2st,
        ):
            # ---- prologue ----
            u_sb = misc.tile([128, KT1, D], bf16)
            m_sb = misc.tile([128, KT1, 128], bf16)
            qs = [0, 14, 28, 41, KT1]
            for qi in range(4):
                lo, hi = qs[qi], qs[qi + 1]
                nc.sync.dma_start(out=m_sb[:, lo:hi, :], in_=m_tab.ap()[:, lo * 128:hi * 128])
                nc.sync.dma_start(out=u_sb[:, lo:hi, :], in_=u_tab.ap()[:, lo * D:hi * D])
            emb_sb = misc.tile([128, D], f32)
            nc.scalar.dma_start(out=emb_sb[:, :], in_=emb.ap())
            iota_t = misc.tile([128, HS], fp16)
            nc.scalar.dma_start(out=iota_t[:, :], in_=iota.ap())
            ses_t = misc.tile([128, 2], f32)
            nc.scalar.dma_start(out=ses_t[:, :], in_=se_s.ap())
            see_t = misc.tile([128, 2], f32)
            nc.scalar.dma_start(out=see_t[:, :], in_=se_e.ap())

            # ---- block one-hot: ohb[t, b] = (b >= Sb_t) & (b < Eb_t) ----
            ohb = misc.tile([128, NB], fp16, tag="ohb")
            for half in range(2):
                t1 = misc.tile([128, HS], fp16, tag=f"t1_{half}")
                nc.vector.tensor_scalar(
                    t1[:, :], iota_t[:, :], ses_t[:, half:half + 1], None,
                    op0=mybir.AluOpType.is_ge,
                )
                t2 = misc.tile([128, HS], fp16, tag=f"t2_{half}")
                nc.vector.tensor_scalar(
                    t2[:, :], iota_t[:, :], see_t[:, half:half + 1], None,
                    op0=mybir.AluOpType.is_ge,
                )
                nc.vector.tensor_tensor(
                    ohb[:, half * HS:(half + 1) * HS], t1[:, :], t2[:, :],
                    op=mybir.AluOpType.subtract,
                )

            # ---- stage 1: type_attr_sum = M @ U (accumulating bf16 matmul) ----
            pse = s2ps.tile([128, 8, D], f32, tag="pp", name="ps1")
            ps1 = pse[:, 0, :]
            for j in range(KT1):
                nc.tensor.matmul(
                    ps1, m_sb[:, j, :], u_sb[:, j, :],
                    start=(j == 0), stop=(j == KT1 - 1),
                )
            chi = misc.tile([128, D], fp16, tag="chi")
            nc.vector.tensor_tensor(
                chi[:, :], ps1, emb_sb[:, :], op=mybir.AluOpType.add,
            )

            # ---- stage 2: gather one row per block, then replicate via DMA ----
            bt = misc.tile([128, NTILE, D], fp16, tag="bt")
            for g in range(NTILE // 8):
                pp = s2ps.tile([128, 8, D], f32, tag="pp", name=f"pp{g}")
                for h in range(8):
                    u = g * 8 + h
                    nc.tensor.matmul(
                        pp[:, h, :],
                        ohb[:, u * 128:(u + 1) * 128],
                        chi[:, :],
                        start=True, stop=True,
                    )
                nc.scalar.copy(bt[:, g * 8:g * 8 + 5, :], pp[:, :5, :])
                nc.vector.tensor_copy(bt[:, g * 8 + 5:g * 8 + 8, :], pp[:, 5:, :])

            # replicate each block row 32x in SBUF by log-doubling (every copy
            # has a contiguous step-1 source -> fast DVE mode; stride-0-src
            # DMA measured as 512B packets, too slow), then write 16KB packets
            for j in range(NTILE):
                st = s2st.tile([128, BLK, D], fp16, tag="st", name=f"st{j}")
                nc.vector.tensor_copy(st[:, 0:1, :], bt[:, j:j + 1, :])
                nc.vector.tensor_copy(st[:, 1:2, :], st[:, 0:1, :])
                nc.vector.tensor_copy(st[:, 2:4, :], st[:, 0:2, :])
                nc.vector.tensor_copy(st[:, 4:8, :], st[:, 0:4, :])
                nc.vector.tensor_copy(st[:, 8:16, :], st[:, 0:8, :])
                nc.vector.tensor_copy(st[:, 16:24, :], st[:, 0:8, :])
                nc.scalar.copy(st[:, 24:32, :], st[:, 8:16, :])
                dst_ap = bass.AP(out_dev, j * 128 * BLK * D,
                                 [[BLK * D, 128], [D, BLK], [1, D]])
                nc.sync.dma_start(out=dst_ap, in_=st[:, :, :])

    nc.compile()
    _cached["nc"] = nc
    return nc


def _prep_in_maps(data, attr_table, edge_type_embedding, flat_attr_ids, attr_seg_ids):
    import ml_dtypes
    bf16 = ml_dtypes.bfloat16

    ids = np.asarray(flat_attr_ids).astype(np.int64)
    segs = np.asarray(attr_seg_ids).astype(np.int64)
    data = np.asarray(data).astype(np.int64)
    attr_table = np.ascontiguousarray(np.asarray(attr_table, dtype=np.float32))
    edge_emb = np.asarray(edge_type_embedding, dtype=np.float32)

    iota = np.broadcast_to(np.arange(HS, dtype=np.float16)[None, :], (128, HS))

    in_maps = []
    edge_rows = []
    for k in range(NCORES):
        # ---- stage 1: compact dedup table + count matrix for this core ----
        own = (segs // TPC) == k
        tloc = segs[own] - TPC * k                       # 0..124
        uniq, inv = np.unique(ids[own], return_inverse=True)
        nu = len(uniq)
        assert nu <= NU_PAD, f"core {k}: {nu} unique attr rows > {NU_PAD}"
        U = np.zeros((NU_PAD, D), np.float32)
        U[:nu] = attr_table[uniq]
        M = np.zeros((128, NU_PAD), np.float32)
        np.add.at(M, (tloc, inv), 1.0)
        u_tab = np.ascontiguousarray(
            U.reshape(KT1, 128, D).transpose(1, 0, 2).reshape(128, KT1 * D).astype(bf16))
        m_tab = np.ascontiguousarray(
            M.T.reshape(KT1, 128, 128).transpose(1, 0, 2).reshape(128, KT1 * 128).astype(bf16))

        emb_k = np.zeros((128, D), np.float32)
        emb_k[:TPC] = edge_emb[TPC * k:TPC * (k + 1)]

        # ---- stage 2: 32-row single-type blocks ----
        sel = np.nonzero((data // TPC) == k)[0]
        dv = (data[sel] - TPC * k).astype(np.int64)      # 0..124
        cnt = np.bincount(dv, minlength=TPC)             # edges per type
        nblk = (cnt + BLK - 1) // BLK                    # blocks per type
        assert nblk.sum() <= NB, f"core {k}: {nblk.sum()} blocks > {NB}"
        Sb = np.zeros(128, np.int64)
        Eb = np.zeros(128, np.int64)
        Sb[:TPC] = np.concatenate(([0], np.cumsum(nblk)[:-1]))
        Eb[:TPC] = np.cumsum(nblk)
        # edge -> output row: padded run start + rank within type
        run_start = Sb[:TPC] * BLK                       # padded row starts
        cum = np.concatenate(([0], np.cumsum(cnt)[:-1]))  # unpadded starts
        order = np.argsort(dv, kind="stable")
        ranks = np.arange(dv.shape[0]) - cum[dv[order]]  # rank within type (sorted)
        rows = np.empty(dv.shape[0], np.int64)
        rows[order] = run_start[dv[order]] + ranks
        edge_rows.append((sel, rows))

        se_s_arr = np.stack([np.clip(Sb, 0, HS), np.clip(Sb - HS, 0, HS)], axis=1)
        se_e_arr = np.stack([np.clip(Eb, 0, HS), np.clip(Eb - HS, 0, HS)], axis=1)

        in_maps.append({
            "u_tab": u_tab,
            "m_tab": m_tab,
            "emb": emb_k,
            "iota": np.ascontiguousarray(iota),
            "se_s": np.ascontiguousarray(se_s_arr.astype(np.float32)),
            "se_e": np.ascontiguousarray(se_e_arr.astype(np.float32)),
        })
    return in_maps, edge_rows


def run(inputs, trace=False, trace_cores=None):
    nc = _build_program()
    in_maps, edge_rows = _prep_in_maps(**inputs)
    kwargs = {}
    if trace:
        kwargs = dict(trace=True)
        if trace_cores is not None:
            kwargs["trace_cores"] = trace_cores
    res = run_bass_kernel_spmd(nc, in_maps, core_ids=list(range(NCORES)), **kwargs)
    outp = np.empty((N, D), np.float32)
    for k in range(NCORES):
        sel, rows = edge_rows[k]
        outp[sel] = res.results[k]["out_dev"][rows].astype(np.float32)
    return outp, res


def kernel(**inputs) -> np.ndarray:
    outp, _ = run(inputs, trace=False)
    return outp
